# revision 25
# baseline (speedup 1.0000x reference)
"""Trainium2 Bass kernel for GQA attention (B=1, S=2048, D=2048, 32 Q heads,
8 KV heads, head_dim 64), 8-way tensor parallel over heads.

Strategy (SPMD, one graph on all 8 cores):
  - Core c owns Q heads 4c..4c+3 and KV head c (GQA maps exactly).
  - Host prep: x transposed to model-dim-major bf16, chunked [128,1024] so
    projections start after ~2MB lands; weight slices bf16; RoPE pairs
    de-interleaved via weight-column permutation; cos/sin tables and windowed
    multiplicative mask tiles built from the actual mask input.
  - Projections n-chunk-outer with attention for q-group g=n interleaved in
    emission order so exp (ScalarE) overlaps projection matmuls (PE).
  - RoPE partition block-swap via a PE permutation matmul (no SBUF-SBUF DMA).
  - Scores for a head pair run as two concurrent row-tiled matmuls
    (tile_position (0,0)/(64,0), K=64 each) sharing the PE array; the rot
    output layout (two heads stacked in 128 partitions) feeds this directly.
  - Causal trimming: per (g, k'-tile) only the non-fully-masked q-column
    window is computed through scores/exp/PV.
  - attn^T = [V|1]^T P^T accumulated in PSUM (ones column = softmax denom),
    scaled by reciprocal, AllToAll per head pair; wo accumulates both halves
    into PSUM held across the second A2A.
  - DMA issue is spread across engine queues: bulk x + wo on sync (wo slots
    alias the consumed xt slots), weights on scalar/vector, a2a traffic and
    consts on gpsimd, outputs on vector.
"""

import os
import sys

import numpy as np

for _p in ("/opt/trn_rl_repo", "/root/.axon_site/_ro/trn_rl_repo"):
    if os.path.isdir(_p) and _p not in sys.path:
        sys.path.insert(0, _p)

import ml_dtypes  # noqa: E402

from concourse import bacc, mybir, tile  # noqa: E402
from concourse.bass_utils import run_bass_kernel_spmd  # noqa: E402

BF16 = mybir.dt.bfloat16
F32 = mybir.dt.float32

S = 2048          # sequence length
D = 2048          # model dim
HD = 64           # head dim
NH = 32           # query heads
NKV = 8           # kv heads
NC = 8            # cores
HL = NH // NC     # q heads per core = 4
P = 128
QG = 512          # q-group width (score-tile free dim)
NG = S // QG      # 4 q groups
NT = S // P       # 16 k'-tiles
KD = D // P       # 16 contraction tiles for D-reductions
SR = S // NC      # 256 output rows per core
XW = 1024         # xt DMA chunk width

_bf = ml_dtypes.bfloat16


def _classify_mask(mask):
    """Per (q-group g, k'-tile t) of the transposed multiplicative mask
    exp(mask)[k, q]: either skipped entirely (None), or (off, mw, u): active
    q-column window [off, 512) with a multiply window [off, off+mw) using
    unique windowed mask tile u (u = -1 if no multiply needed)."""
    mexp = np.exp(np.minimum(mask.astype(np.float64), 50.0)).astype(np.float32).T
    kinds = {}
    uniq = []
    uniq_keys = {}
    for g in range(NG):
        for t in range(NT):
            tl = mexp[P * t:P * (t + 1), QG * g:QG * (g + 1)]
            zero_col = (tl == 0.0).all(axis=0)
            one_col = (tl == 1.0).all(axis=0)
            if zero_col.all():
                kinds[(g, t)] = None
                continue
            nz = np.where(~zero_col)[0]
            off = (int(nz[0]) // 8) * 8
            ntriv = np.where(~one_col)[0]
            if len(ntriv) == 0 and off == 0:
                kinds[(g, t)] = (0, 0, -1)
                continue
            last = int(ntriv[-1]) + 1 if len(ntriv) else off + 1
            m1 = min(QG, -(-last // 8) * 8)
            win = tl[:, off:m1]
            key = (m1 - off, win.tobytes())
            if key not in uniq_keys:
                uniq_keys[key] = len(uniq)
                uniq.append(win.astype(_bf))
            kinds[(g, t)] = (off, m1 - off, uniq_keys[key])
    # the first active tile of each group must start at column 0 so its PV
    # matmul initializes the whole PSUM accumulator width
    for g in range(NG):
        for t in range(NT):
            k = kinds.get((g, t))
            if k is None:
                continue
            off, mw, u = k
            if off != 0:
                tl = mexp[P * t:P * (t + 1), QG * g:QG * (g + 1)]
                m1 = off + mw if u >= 0 else off
                m1 = max(m1, (off // 8 + 1) * 8)
                win = tl[:, 0:m1]
                key = (m1, win.tobytes())
                if key not in uniq_keys:
                    uniq_keys[key] = len(uniq)
                    uniq.append(win.astype(_bf))
                kinds[(g, t)] = (0, m1, uniq_keys[key])
            break
    return kinds, uniq


def _build_nc(kinds, uniq_widths):
    n_uniq = len(uniq_widths)
    mw_max = max(uniq_widths) if n_uniq else 0
    nc = bacc.Bacc("TRN2", target_bir_lowering=False, debug=False,
                   num_devices=NC)

    xt_d = nc.dram_tensor("xt", [D, S], BF16, kind="ExternalInput")
    wq_d = nc.dram_tensor("wq", [D, HL * HD], BF16, kind="ExternalInput")
    wkv_d = nc.dram_tensor("wkv", [D, 2 * HD], BF16, kind="ExternalInput")
    wo_d = nc.dram_tensor("wo", [D, D], BF16, kind="ExternalInput")
    cos2_d = nc.dram_tensor("cos2", [P, S], BF16, kind="ExternalInput")
    sin2_d = nc.dram_tensor("sin2", [P, S], BF16, kind="ExternalInput")
    ident_d = nc.dram_tensor("ident", [P, P], BF16, kind="ExternalInput")
    perm_d = nc.dram_tensor("perm", [P, P], BF16, kind="ExternalInput")
    mt_d = None
    if n_uniq:
        mt_d = nc.dram_tensor("mtiles", [n_uniq, P, mw_max], BF16,
                              kind="ExternalInput")
    out_d = nc.dram_tensor("out", [SR, D], F32, kind="ExternalOutput")

    NXC = S // XW  # xt column chunks per k-slice

    with tile.TileContext(nc) as tc:
        with (
            tc.tile_pool(name="big", bufs=1) as big,        # xt chunks, then wo
            tc.tile_pool(name="wp", bufs=1) as wp,          # weights/consts
            tc.tile_pool(name="work", bufs=2) as work,
            tc.tile_pool(name="persist", bufs=1) as persist,
            tc.tile_pool(name="pt", bufs=6) as ptpool,
            tc.tile_pool(name="ps_sc", bufs=2, space="PSUM") as ps_sc,   # [128,2,512] f32 = 2 banks ea
            tc.tile_pool(name="ps_at", bufs=4, space="PSUM") as ps_at,   # 1 bank ea
            tc.tile_pool(name="dram", bufs=1, space="DRAM") as dram,
        ):
            # ---- input DMAs, spread across engine queues ----
            wq_sb = [wp.tile([P, HL * HD], BF16, tag=f"wq{k}", name=f"wq{k}")
                     for k in range(KD)]
            wkv_sb = [wp.tile([P, 2 * HD], BF16, tag=f"wkv{k}", name=f"wkv{k}")
                      for k in range(KD)]
            for k in range(KD):
                nc.gpsimd.dma_start(wkv_sb[k][:], wkv_d.ap()[P * k:P * (k + 1), :])
            cos2 = wp.tile([P, S], BF16)
            sin2 = wp.tile([P, S], BF16)
            ident = wp.tile([P, P], BF16)
            perm = wp.tile([P, P], BF16)
            nc.gpsimd.dma_start(cos2[:], cos2_d.ap())
            nc.gpsimd.dma_start(sin2[:], sin2_d.ap())
            nc.gpsimd.dma_start(ident[:], ident_d.ap())
            nc.gpsimd.dma_start(perm[:], perm_d.ap())
            mt = None
            if n_uniq:
                mt = wp.tile([P, n_uniq, 2, mw_max], BF16)
                for u in range(n_uniq):
                    # duplicated per head-of-pair for one-shot TT masks
                    for h2 in range(2):
                        nc.gpsimd.dma_start(mt[:, u, h2, 0:uniq_widths[u]],
                                            mt_d.ap()[u])
            # xt chunks split across sync/scalar queues, n-major so chunk 0
            # lands first; wq joins scalar after the first xt wave
            xt_sb = [[big.tile([P, XW], BF16, tag=f"big{NXC * k + j}",
                               name=f"x{k}_{j}")
                      for j in range(NXC)] for k in range(KD)]

            def xt_load(j):
                for k in range(KD):
                    eng = nc.sync if k % 2 == 0 else nc.scalar
                    eng.dma_start(xt_sb[k][j][:],
                                  xt_d.ap()[P * k:P * (k + 1),
                                            XW * j:XW * (j + 1)])

            xt_load(0)
            for k in range(KD):
                nc.scalar.dma_start(wq_sb[k][:], wq_d.ap()[P * k:P * (k + 1), :])
            xt_load(1)
            # wo reuses the xt slots (slot k waits for xt tile k's release)
            wo_sb = [big.tile([P, D], BF16, tag=f"big{NXC * k}", name=f"wo{k}")
                     for k in range(KD)]
            for k in range(KD):
                nc.sync.dma_start(wo_sb[k][:], wo_d.ap()[P * k:P * (k + 1), :])

            # ---- persistent activations ----
            qrot = persist.tile([P, 2, S], BF16)   # pair m: rows 0-63 head 2m, 64-127 head 2m+1
            krot2 = persist.tile([P, S], BF16)     # K rot duplicated on both halves
            vt_sb = persist.tile([HD, S], BF16)
            v_sb = persist.tile([P, NT, HD + 2], BF16)  # col 64 = ones
            nc.vector.memset(v_sb[:, :, HD:HD + 2], 1.0)

            a2a_in = dram.tile([NC, HL * HD, SR], BF16, name="a2ai")
            a2a_out = dram.tile([NC, HL * HD, SR], BF16, name="a2ao")

            def proj_units(n, m):
                """Emission units for projection block (n, m): 4 matmul
                quads + a RoPE finisher (m=0,1: q head pairs; m=2: K|V)."""
                nsl = slice(QG * n, QG * (n + 1))
                j, joff = (QG * n) // XW, (QG * n) % XW
                st = {}

                def mm_quad(kk):
                    def u():
                        if kk == 0:
                            st["ps"] = ps_at.tile([P, QG], F32, tag="at",
                                                  name=f"pj{n}{m}")
                        for k in range(4 * kk, 4 * kk + 4):
                            lhsT = (wq_sb[k][:, P * m:P * (m + 1)] if m < 2
                                    else wkv_sb[k][:])
                            nc.tensor.matmul(st["ps"][:], lhsT,
                                             xt_sb[k][j][:, joff:joff + QG],
                                             start=(k == 0),
                                             stop=(k == KD - 1))
                    return u

                def rope():
                    ps = st["ps"]
                    raw = work.tile([P, QG], BF16, tag="raw")
                    nc.vector.tensor_copy(raw[:], ps[:])
                    # partition block-swap via PE permutation matmul
                    psw = ps_at.tile([P, QG], F32, tag="at", name=f"pw{n}{m}")
                    nc.tensor.matmul(psw[:], perm[:], raw[:],
                                     start=True, stop=True)
                    t1 = work.tile([P, QG], BF16, tag="t1")
                    nc.vector.tensor_mul(t1[:], raw[:], cos2[:, nsl])
                    t2 = work.tile([P, QG], BF16, tag="t2")
                    nc.vector.tensor_mul(t2[:], psw[:], sin2[:, nsl])
                    if m < 2:
                        nc.vector.tensor_add(qrot[:, m, nsl], t1[:], t2[:])
                    else:
                        rot = work.tile([P, QG], BF16, tag="rot")
                        nc.vector.tensor_add(rot[:], t1[:], t2[:])
                        nc.vector.tensor_copy(krot2[0:HD, nsl], rot[0:HD, :])
                        # duplicate K rot onto partitions 64-127
                        nc.gpsimd.dma_start(krot2[HD:P, nsl], rot[0:HD, :])
                        nc.scalar.dma_start(vt_sb[:, nsl], raw[HD:P, :])

                return [mm_quad(kk) for kk in range(4)] + [rope]

            def vtrans_units(n):
                """V-transpose units for chunk n (after its m=2 rope)."""
                def tr(t):
                    def u():
                        pv = ps_at.tile([P, HD], BF16, tag="at", name=f"pv{t}")
                        nc.tensor.transpose(pv[:], vt_sb[:, P * t:P * (t + 1)],
                                            ident[0:HD, 0:HD])
                        nc.vector.tensor_copy(v_sb[:, t, 0:HD], pv[:])
                    return u
                return [tr(t) for t in range(4 * n, 4 * n + 4)]

            def attn_units(hp, g):
                """Attention units for head pair hp on q-group g: one per
                k'-tile plus a normalize/store finisher."""
                tiles = [t for t in range(NT) if kinds.get((g, t)) is not None]
                qb = QG * g
                st = {}

                def tile_unit(ci, t):
                    def u():
                        if ci == 0:
                            st["att"] = [
                                ps_at.tile([HD + 1, QG], F32, tag="at",
                                           name=f"att{hp}{g}{h2}")
                                for h2 in range(2)]
                        off, mw, mu = kinds[(g, t)]
                        w = QG - off
                        psc = ps_sc.tile([P, 2, QG], F32, tag="sc",
                                         name=f"sc{hp}{g}{t}")
                        for h2 in range(2):
                            nc.tensor.matmul(
                                psc[:, h2, 0:w],
                                krot2[HD * h2:HD * (h2 + 1), P * t:P * (t + 1)],
                                qrot[HD * h2:HD * (h2 + 1), hp,
                                     qb + off:qb + QG],
                                start=True, stop=True)
                        pts = ptpool.tile([P, 2, QG], BF16, tag="pt")
                        nc.scalar.activation(pts[:, :, 0:w], psc[:, :, 0:w],
                                             mybir.ActivationFunctionType.Exp,
                                             scale=0.125)
                        if mu >= 0:
                            nc.vector.tensor_mul(pts[:, :, 0:mw],
                                                 pts[:, :, 0:mw],
                                                 mt[:, mu, :, 0:mw])
                        for h2 in range(2):
                            nc.tensor.matmul(st["att"][h2][:, off:QG],
                                             v_sb[:, t, 0:HD + 1],
                                             pts[:, h2, 0:w],
                                             start=(ci == 0),
                                             stop=(ci == len(tiles) - 1),
                                             skip_group_check=True)
                    return u

                def finish():
                    for h2 in range(2):
                        den = work.tile([1, QG], F32, tag="den")
                        nc.vector.tensor_copy(den[:], st["att"][h2][HD:HD + 1, :])
                        rec = work.tile([1, QG], F32, tag="rec")
                        nc.vector.reciprocal_approx_fast(rec[:], den[:])
                        rec64 = work.tile([HD, QG], F32, tag="rec64")
                        nc.gpsimd.partition_broadcast(rec64[:], rec[:])
                        asb = work.tile([HD, QG], BF16, tag="asb")
                        nc.vector.tensor_mul(asb[:], st["att"][h2][0:HD, :],
                                             rec64[:])
                        hr = HD * (2 * hp + h2)
                        nc.gpsimd.dma_start(a2a_in[2 * g, hr:hr + HD, :],
                                            asb[:, 0:SR])
                        nc.gpsimd.dma_start(a2a_in[2 * g + 1, hr:hr + HD, :],
                                            asb[:, SR:2 * SR])

                return [tile_unit(ci, t) for ci, t in enumerate(tiles)] + [finish]

            def zipper(A, B):
                """Emit A and B interleaved in proportional order."""
                ia = ib = 0
                while ia < len(A) or ib < len(B):
                    if ib < len(B) and (ia >= len(A)
                                        or ib * len(A) <= ia * len(B)):
                        B[ib]()
                        ib += 1
                    else:
                        A[ia]()
                        ia += 1

            # ---- emission: attention for q-group n interleaved with the
            # projections of chunk n+1, so PE fills exp-wait gaps ----
            for u in (proj_units(0, 2) + vtrans_units(0) + proj_units(0, 0)
                      + proj_units(0, 1)):
                u()
            for n in range(NG):
                A = attn_units(0, n) + attn_units(1, n)
                B = []
                if n + 1 < NG:
                    B = (proj_units(n + 1, 2) + vtrans_units(n + 1)
                         + proj_units(n + 1, 0) + proj_units(n + 1, 1))
                zipper(A, B)
            nc.gpsimd.collective_compute(
                "AllToAll", mybir.AluOpType.bypass,
                replica_groups=[list(range(NC))],
                ins=[a2a_in.opt()], outs=[a2a_out.opt()])

            # ---- output: out[s_rows, :] = attnT_full^T @ wo ----
            # a2a_out block i rows [128p, 128p+128) = wo k-tile 2i+p
            ao_sb = [persist.tile([P, SR], BF16, tag=f"ao{k}", name=f"ao{k}")
                     for k in range(KD)]
            for k in range(KD):
                nc.gpsimd.dma_start(ao_sb[k][:],
                                    a2a_out[k // 2, P * (k % 2):P * (k % 2 + 1), :])

            # 8 accumulators [128, 512] over all 8 PSUM banks; k-inner with
            # 4 output-column matmuls per stationary load.
            NE = D // QG
            acc = {}
            for sm in range(SR // P):
                pscl = ps_sc.tile([P, 2, QG], F32, tag="sc", name=f"oacc{sm}")
                acc[(sm, 0)] = pscl[:, 0, :]
                acc[(sm, 1)] = pscl[:, 1, :]
                for e2 in range(2):
                    a = ps_at.tile([P, QG], F32, tag="at", name=f"oat{sm}{e2}")
                    acc[(sm, 2 + e2)] = a[:]
            for sm in range(SR // P):
                for k in range(KD):
                    for ec in range(NE):
                        nc.tensor.matmul(
                            acc[(sm, ec)],
                            ao_sb[k][:, P * sm:P * (sm + 1)],
                            wo_sb[k][:, QG * ec:QG * (ec + 1)],
                            start=(k == 0), stop=(k == KD - 1))
                for ec in range(NE):
                    osb = work.tile([P, QG], F32, tag="osb")
                    nc.vector.tensor_copy(osb[:], acc[(sm, ec)])
                    eng = nc.scalar if ec % 2 == 0 else nc.sync
                    eng.dma_start(
                        out_d.ap()[P * sm:P * (sm + 1),
                                   QG * ec:QG * (ec + 1)],
                        osb[:])

    nc.compile()
    return nc


_CACHE = {}


def _get_compiled(mask):
    kinds, uniq = _classify_mask(mask)
    key = tuple(sorted(kinds.items())) + tuple(u.tobytes() for u in uniq)
    if key not in _CACHE:
        _CACHE[key] = _build_nc(kinds, [u.shape[1] for u in uniq])
    return _CACHE[key], kinds, uniq


def _host_prep(x, freqs_cos, freqs_sin, mask, wq, wk, wv, wo, uniq):
    xt = np.ascontiguousarray(x[0].T).astype(_bf)
    perm_cols = np.concatenate([np.arange(0, HD, 2), np.arange(1, HD, 2)])
    cosT = np.ascontiguousarray(freqs_cos.T)            # [32, S]
    sinT = np.ascontiguousarray(freqs_sin.T)
    cos2 = np.tile(cosT, (4, 1)).astype(_bf)            # [128, S]
    sin2 = np.tile(np.concatenate([-sinT, sinT], axis=0), (2, 1)).astype(_bf)
    ident = np.eye(P, dtype=_bf)
    # block-swap permutation matrix: swap the 32-halves within each 64 block
    sw = np.zeros((P, P), dtype=_bf)
    for b in range(4):
        d0, s0 = 32 * b, 32 * (b ^ 1)
        sw[s0:s0 + 32, d0:d0 + 32] = np.eye(32, dtype=_bf)
    wo_b = np.ascontiguousarray(wo).astype(_bf)
    mt = None
    if uniq:
        mw_max = max(u.shape[1] for u in uniq)
        mt = np.zeros((len(uniq), P, mw_max), dtype=_bf)
        for i, u in enumerate(uniq):
            mt[i, :, :u.shape[1]] = u
    in_maps = []
    for c in range(NC):
        qcols = np.concatenate(
            [HD * (HL * c + h) + perm_cols for h in range(HL)])
        wq_c = np.ascontiguousarray(wq[:, qcols]).astype(_bf)
        wkv_c = np.concatenate(
            [wk[:, HD * c + perm_cols], wv[:, HD * c:HD * (c + 1)]],
            axis=1).astype(_bf)
        m = {"xt": xt, "wq": wq_c, "wkv": np.ascontiguousarray(wkv_c),
             "wo": wo_b, "cos2": cos2, "sin2": sin2, "ident": ident,
             "perm": sw}
        if mt is not None:
            m["mtiles"] = mt
        in_maps.append(m)
    return in_maps


def run(x, freqs_cos, freqs_sin, mask, wq, wk, wv, wo, trace=False):
    x = np.asarray(x, dtype=np.float32)
    mask = np.asarray(mask, dtype=np.float32)
    nc, kinds, uniq = _get_compiled(mask)
    in_maps = _host_prep(x, np.asarray(freqs_cos), np.asarray(freqs_sin),
                         mask, np.asarray(wq), np.asarray(wk),
                         np.asarray(wv), np.asarray(wo), uniq)
    res = run_bass_kernel_spmd(nc, in_maps, core_ids=list(range(NC)),
                               trace=trace)
    out = np.concatenate([res.results[c]["out"] for c in range(NC)], axis=0)
    return out.reshape(1, S, D).astype(np.float32), res


def kernel(x, freqs_cos, freqs_sin, mask, wq, wk, wv, wo):
    out, _ = run(x, freqs_cos, freqs_sin, mask, wq, wk, wv, wo, trace=False)
    return out


# revision 28
# speedup vs baseline: 1.0575x; 1.0575x over previous
"""Trainium2 Bass kernel for GQA attention (B=1, S=2048, D=2048, 32 Q heads,
8 KV heads, head_dim 64), 8-way tensor parallel over heads.

Strategy (SPMD, one graph on all 8 cores):
  - Core c owns Q heads 4c..4c+3 and KV head c (GQA maps exactly).
  - Host prep: x transposed to model-dim-major bf16, chunked [128,1024] so
    projections start after ~2MB lands; weight slices bf16; RoPE pairs
    de-interleaved via weight-column permutation; cos/sin tables and windowed
    multiplicative mask tiles built from the actual mask input.
  - Projections n-chunk-outer with attention for q-group g=n interleaved in
    emission order so exp (ScalarE) overlaps projection matmuls (PE).
  - RoPE partition block-swap via a PE permutation matmul (no SBUF-SBUF DMA).
  - Scores for a head pair run as two concurrent row-tiled matmuls
    (tile_position (0,0)/(64,0), K=64 each) sharing the PE array; the rot
    output layout (two heads stacked in 128 partitions) feeds this directly.
  - Causal trimming: per (g, k'-tile) only the non-fully-masked q-column
    window is computed through scores/exp/PV.
  - attn^T = [V|1]^T P^T accumulated in PSUM (ones column = softmax denom),
    scaled by reciprocal, AllToAll per head pair; wo accumulates both halves
    into PSUM held across the second A2A.
  - DMA issue is spread across engine queues: bulk x + wo on sync (wo slots
    alias the consumed xt slots), weights on scalar/vector, a2a traffic and
    consts on gpsimd, outputs on vector.
"""

import os
import sys

import numpy as np

for _p in ("/opt/trn_rl_repo", "/root/.axon_site/_ro/trn_rl_repo"):
    if os.path.isdir(_p) and _p not in sys.path:
        sys.path.insert(0, _p)

import ml_dtypes  # noqa: E402

from concourse import bacc, mybir, tile  # noqa: E402
from concourse.bass_utils import run_bass_kernel_spmd  # noqa: E402

BF16 = mybir.dt.bfloat16
F32 = mybir.dt.float32

S = 2048          # sequence length
D = 2048          # model dim
HD = 64           # head dim
NH = 32           # query heads
NKV = 8           # kv heads
NC = 8            # cores
HL = NH // NC     # q heads per core = 4
P = 128
QG = 512          # q-group width (score-tile free dim)
NG = S // QG      # 4 q groups
NT = S // P       # 16 k'-tiles
KD = D // P       # 16 contraction tiles for D-reductions
SR = S // NC      # 256 output rows per core
XW = 1024         # xt DMA chunk width

_bf = ml_dtypes.bfloat16


def _classify_mask(mask):
    """Per (q-group g, k'-tile t) of the transposed multiplicative mask
    exp(mask)[k, q]: either skipped entirely (None), or (off, mw, u): active
    q-column window [off, 512) with a multiply window [off, off+mw) using
    unique windowed mask tile u (u = -1 if no multiply needed)."""
    mexp = np.exp(np.minimum(mask.astype(np.float64), 50.0)).astype(np.float32).T
    kinds = {}
    uniq = []
    uniq_keys = {}
    for g in range(NG):
        for t in range(NT):
            tl = mexp[P * t:P * (t + 1), QG * g:QG * (g + 1)]
            zero_col = (tl == 0.0).all(axis=0)
            one_col = (tl == 1.0).all(axis=0)
            if zero_col.all():
                kinds[(g, t)] = None
                continue
            nz = np.where(~zero_col)[0]
            off = (int(nz[0]) // 8) * 8
            ntriv = np.where(~one_col)[0]
            if len(ntriv) == 0 and off == 0:
                kinds[(g, t)] = (0, 0, -1)
                continue
            last = int(ntriv[-1]) + 1 if len(ntriv) else off + 1
            m1 = min(QG, -(-last // 8) * 8)
            win = tl[:, off:m1]
            key = (m1 - off, win.tobytes())
            if key not in uniq_keys:
                uniq_keys[key] = len(uniq)
                uniq.append(win.astype(_bf))
            kinds[(g, t)] = (off, m1 - off, uniq_keys[key])
    # the first active tile of each group must start at column 0 so its PV
    # matmul initializes the whole PSUM accumulator width
    for g in range(NG):
        for t in range(NT):
            k = kinds.get((g, t))
            if k is None:
                continue
            off, mw, u = k
            if off != 0:
                tl = mexp[P * t:P * (t + 1), QG * g:QG * (g + 1)]
                m1 = off + mw if u >= 0 else off
                m1 = max(m1, (off // 8 + 1) * 8)
                win = tl[:, 0:m1]
                key = (m1, win.tobytes())
                if key not in uniq_keys:
                    uniq_keys[key] = len(uniq)
                    uniq.append(win.astype(_bf))
                kinds[(g, t)] = (0, m1, uniq_keys[key])
            break
    return kinds, uniq


def _build_nc(kinds, uniq_widths):
    n_uniq = len(uniq_widths)
    mw_max = max(uniq_widths) if n_uniq else 0
    nc = bacc.Bacc("TRN2", target_bir_lowering=False, debug=False,
                   num_devices=NC)

    xt_d = nc.dram_tensor("xt", [D, S], BF16, kind="ExternalInput")
    wq_d = nc.dram_tensor("wq", [D, HL * HD], BF16, kind="ExternalInput")
    wkv_d = nc.dram_tensor("wkv", [D, 2 * HD], BF16, kind="ExternalInput")
    wo_d = nc.dram_tensor("wo", [D, D], BF16, kind="ExternalInput")
    cos2_d = nc.dram_tensor("cos2", [P, S], BF16, kind="ExternalInput")
    sin2_d = nc.dram_tensor("sin2", [P, S], BF16, kind="ExternalInput")
    ident_d = nc.dram_tensor("ident", [P, P], BF16, kind="ExternalInput")
    perm_d = nc.dram_tensor("perm", [P, P], BF16, kind="ExternalInput")
    mt_d = None
    if n_uniq:
        mt_d = nc.dram_tensor("mtiles", [n_uniq, P, mw_max], BF16,
                              kind="ExternalInput")
    out_d = nc.dram_tensor("out", [SR, D], BF16, kind="ExternalOutput")

    NXC = S // XW  # xt column chunks per k-slice

    with tile.TileContext(nc) as tc:
        with (
            tc.tile_pool(name="big", bufs=1) as big,        # xt chunks, then wo
            tc.tile_pool(name="wp", bufs=1) as wp,          # weights/consts
            tc.tile_pool(name="work", bufs=2) as work,
            tc.tile_pool(name="persist", bufs=1) as persist,
            tc.tile_pool(name="pt", bufs=6) as ptpool,
            tc.tile_pool(name="ps_sc", bufs=2, space="PSUM") as ps_sc,   # [128,2,512] f32 = 2 banks ea
            tc.tile_pool(name="ps_at", bufs=4, space="PSUM") as ps_at,   # 1 bank ea
            tc.tile_pool(name="dram", bufs=1, space="DRAM") as dram,
        ):
            # ---- input DMAs, spread across engine queues ----
            wq_sb = [wp.tile([P, HL * HD], BF16, tag=f"wq{k}", name=f"wq{k}")
                     for k in range(KD)]
            wkv_sb = [wp.tile([P, 2 * HD], BF16, tag=f"wkv{k}", name=f"wkv{k}")
                      for k in range(KD)]
            for k in range(KD):
                nc.gpsimd.dma_start(wkv_sb[k][:], wkv_d.ap()[P * k:P * (k + 1), :])
            cos2 = wp.tile([P, S], BF16)
            sin2 = wp.tile([P, S], BF16)
            ident = wp.tile([P, P], BF16)
            perm = wp.tile([P, P], BF16)
            nc.gpsimd.dma_start(cos2[:], cos2_d.ap())
            nc.gpsimd.dma_start(sin2[:], sin2_d.ap())
            nc.gpsimd.dma_start(ident[:], ident_d.ap())
            nc.gpsimd.dma_start(perm[:], perm_d.ap())
            mt = None
            if n_uniq:
                mt = wp.tile([P, n_uniq, 2, mw_max], BF16)
                for u in range(n_uniq):
                    # duplicated per head-of-pair for one-shot TT masks
                    for h2 in range(2):
                        nc.gpsimd.dma_start(mt[:, u, h2, 0:uniq_widths[u]],
                                            mt_d.ap()[u])
            # xt chunks split across sync/scalar queues, n-major so chunk 0
            # lands first; wq joins scalar after the first xt wave
            xt_sb = [[big.tile([P, XW], BF16, tag=f"big{NXC * k + j}",
                               name=f"x{k}_{j}")
                      for j in range(NXC)] for k in range(KD)]

            def xt_load(j):
                for k in range(KD):
                    eng = nc.sync if k % 2 == 0 else nc.scalar
                    eng.dma_start(xt_sb[k][j][:],
                                  xt_d.ap()[P * k:P * (k + 1),
                                            XW * j:XW * (j + 1)])

            xt_load(0)
            for k in range(KD):
                nc.scalar.dma_start(wq_sb[k][:], wq_d.ap()[P * k:P * (k + 1), :])
            xt_load(1)
            # wo reuses the xt slots (slot k waits for xt tile k's release)
            wo_sb = [big.tile([P, D], BF16, tag=f"big{NXC * k}", name=f"wo{k}")
                     for k in range(KD)]
            for k in range(KD):
                nc.sync.dma_start(wo_sb[k][:], wo_d.ap()[P * k:P * (k + 1), :])

            # ---- persistent activations ----
            qrot = persist.tile([P, 2, S], BF16)   # pair m: rows 0-63 head 2m, 64-127 head 2m+1
            krot2 = persist.tile([P, S], BF16)     # K rot duplicated on both halves
            vt_sb = persist.tile([HD, S], BF16)
            v_sb = persist.tile([P, NT, HD + 2], BF16)  # col 64 = ones
            nc.vector.memset(v_sb[:, :, HD:HD + 2], 1.0)

            a2a_in = dram.tile([NC, HL * HD, SR], BF16, name="a2ai")
            a2a_out = dram.tile([NC, HL * HD, SR], BF16, name="a2ao")

            def proj_chunk(n, m):
                """Projection + RoPE for column chunk n, output block m
                (m=0,1: q head pairs; m=2: K|V)."""
                nsl = slice(QG * n, QG * (n + 1))
                j, joff = (QG * n) // XW, (QG * n) % XW
                ps = ps_sc.tile([P, 2, QG], F32, tag="sc", name=f"pj{n}{m}")
                for k in range(KD):
                    lhsT = (wq_sb[k][:, P * m:P * (m + 1)] if m < 2
                            else wkv_sb[k][:])
                    nc.tensor.matmul(ps[:, 0, :], lhsT,
                                     xt_sb[k][j][:, joff:joff + QG],
                                     start=(k == 0), stop=(k == KD - 1))
                raw = work.tile([P, QG], BF16, tag="raw")
                nc.vector.tensor_copy(raw[:], ps[:, 0, :])
                # partition block-swap via PE permutation matmul, into the
                # second bank of the same score slot
                nc.tensor.matmul(ps[:, 1, :], perm[:], raw[:],
                                 start=True, stop=True)
                t1 = work.tile([P, QG], BF16, tag="t1")
                nc.vector.tensor_mul(t1[:], raw[:], cos2[:, nsl])
                t2 = work.tile([P, QG], BF16, tag="t2")
                nc.vector.tensor_mul(t2[:], ps[:, 1, :], sin2[:, nsl])
                if m < 2:
                    nc.vector.tensor_add(qrot[:, m, nsl], t1[:], t2[:])
                else:
                    rot = work.tile([P, QG], BF16, tag="rot")
                    nc.vector.tensor_add(rot[:], t1[:], t2[:])
                    nc.vector.tensor_copy(krot2[0:HD, nsl], rot[0:HD, :])
                    # duplicate K rot onto partitions 64-127
                    nc.gpsimd.dma_start(krot2[HD:P, nsl], rot[0:HD, :])
                    nc.scalar.dma_start(vt_sb[:, nsl], raw[HD:P, :])
                    for t in range(4 * n, 4 * n + 4):
                        pv = ps_at.tile([P, HD], BF16, tag="at", name=f"pv{t}")
                        nc.tensor.transpose(pv[:], vt_sb[:, P * t:P * (t + 1)],
                                            ident[0:HD, 0:HD])
                        nc.vector.tensor_copy(v_sb[:, t, 0:HD], pv[:])

            def attn_group(hp, g):
                """Attention for head pair hp (heads 2hp, 2hp+1) on q-group g."""
                tiles = [t for t in range(NT) if kinds.get((g, t)) is not None]
                att = [ps_at.tile([HD + 1, QG], F32, tag="at",
                                  name=f"att{hp}{g}{h2}") for h2 in range(2)]
                qb = QG * g
                first = True
                for ci, t in enumerate(tiles):
                    off, mw, u = kinds[(g, t)]
                    w = QG - off
                    psc = ps_sc.tile([P, 2, QG], F32, tag="sc",
                                     name=f"sc{hp}{g}{t}")
                    for h2 in range(2):
                        nc.tensor.matmul(
                            psc[:, h2, 0:w],
                            krot2[HD * h2:HD * (h2 + 1), P * t:P * (t + 1)],
                            qrot[HD * h2:HD * (h2 + 1), hp, qb + off:qb + QG],
                            start=True, stop=True)
                    pts = ptpool.tile([P, 2, QG], BF16, tag="pt")
                    nc.scalar.activation(pts[:, :, 0:w], psc[:, :, 0:w],
                                         mybir.ActivationFunctionType.Exp,
                                         scale=0.125)
                    if u >= 0:
                        nc.vector.tensor_mul(pts[:, :, 0:mw], pts[:, :, 0:mw],
                                             mt[:, u, :, 0:mw])
                    for h2 in range(2):
                        nc.tensor.matmul(att[h2][:, off:QG],
                                         v_sb[:, t, 0:HD + 1],
                                         pts[:, h2, 0:w],
                                         start=first,
                                         stop=(ci == len(tiles) - 1),
                                         skip_group_check=True)
                    first = False
                for h2 in range(2):
                    den = work.tile([1, QG], F32, tag="den")
                    nc.vector.tensor_copy(den[:], att[h2][HD:HD + 1, :])
                    rec = work.tile([1, QG], F32, tag="rec")
                    nc.vector.reciprocal_approx_fast(rec[:], den[:])
                    rec64 = work.tile([HD, QG], F32, tag="rec64")
                    nc.gpsimd.partition_broadcast(rec64[:], rec[:])
                    asb = work.tile([HD, QG], BF16, tag="asb")
                    nc.vector.tensor_mul(asb[:], att[h2][0:HD, :], rec64[:])
                    hr = HD * (2 * hp + h2)
                    nc.gpsimd.dma_start(a2a_in[2 * g, hr:hr + HD, :],
                                        asb[:, 0:SR])
                    nc.gpsimd.dma_start(a2a_in[2 * g + 1, hr:hr + HD, :],
                                        asb[:, SR:2 * SR])

            # ---- interleaved emission: proj chunk n, then attention g=n ----
            for n in range(NG):
                for m in (2, 0, 1):
                    proj_chunk(n, m)
                attn_group(0, n)
                attn_group(1, n)
            nc.gpsimd.collective_compute(
                "AllToAll", mybir.AluOpType.bypass,
                replica_groups=[list(range(NC))],
                ins=[a2a_in.opt()], outs=[a2a_out.opt()])

            # ---- output: out[s_rows, :] = attnT_full^T @ wo ----
            # a2a_out block i rows [128p, 128p+128) = wo k-tile 2i+p
            ao_sb = [persist.tile([P, SR], BF16, tag=f"ao{k}", name=f"ao{k}")
                     for k in range(KD)]
            for k in range(KD):
                nc.gpsimd.dma_start(ao_sb[k][:],
                                    a2a_out[k // 2, P * (k % 2):P * (k % 2 + 1), :])

            # 8 accumulators [128, 512] over all 8 PSUM banks; k-inner with
            # 4 output-column matmuls per stationary load.
            NE = D // QG
            acc = {}
            for sm in range(SR // P):
                pscl = ps_sc.tile([P, 2, QG], F32, tag="sc", name=f"oacc{sm}")
                acc[(sm, 0)] = pscl[:, 0, :]
                acc[(sm, 1)] = pscl[:, 1, :]
                for e2 in range(2):
                    a = ps_at.tile([P, QG], F32, tag="at", name=f"oat{sm}{e2}")
                    acc[(sm, 2 + e2)] = a[:]
            engs = [nc.scalar, nc.sync, nc.gpsimd]
            for i, (sm, ec) in enumerate([(s, e) for s in range(SR // P)
                                          for e in range(NE)]):
                for k in range(KD):
                    nc.tensor.matmul(
                        acc[(sm, ec)],
                        ao_sb[k][:, P * sm:P * (sm + 1)],
                        wo_sb[k][:, QG * ec:QG * (ec + 1)],
                        start=(k == 0), stop=(k == KD - 1))
                osb = work.tile([P, QG], BF16, tag="osb")
                nc.vector.tensor_copy(osb[:], acc[(sm, ec)])
                engs[i % 3].dma_start(
                    out_d.ap()[P * sm:P * (sm + 1),
                               QG * ec:QG * (ec + 1)],
                    osb[:])

    nc.compile()
    return nc


_CACHE = {}


def _get_compiled(mask):
    kinds, uniq = _classify_mask(mask)
    key = tuple(sorted(kinds.items())) + tuple(u.tobytes() for u in uniq)
    if key not in _CACHE:
        _CACHE[key] = _build_nc(kinds, [u.shape[1] for u in uniq])
    return _CACHE[key], kinds, uniq


def _host_prep(x, freqs_cos, freqs_sin, mask, wq, wk, wv, wo, uniq):
    xt = np.ascontiguousarray(x[0].T).astype(_bf)
    perm_cols = np.concatenate([np.arange(0, HD, 2), np.arange(1, HD, 2)])
    cosT = np.ascontiguousarray(freqs_cos.T)            # [32, S]
    sinT = np.ascontiguousarray(freqs_sin.T)
    cos2 = np.tile(cosT, (4, 1)).astype(_bf)            # [128, S]
    sin2 = np.tile(np.concatenate([-sinT, sinT], axis=0), (2, 1)).astype(_bf)
    ident = np.eye(P, dtype=_bf)
    # block-swap permutation matrix: swap the 32-halves within each 64 block
    sw = np.zeros((P, P), dtype=_bf)
    for b in range(4):
        d0, s0 = 32 * b, 32 * (b ^ 1)
        sw[s0:s0 + 32, d0:d0 + 32] = np.eye(32, dtype=_bf)
    wo_b = np.ascontiguousarray(wo).astype(_bf)
    mt = None
    if uniq:
        mw_max = max(u.shape[1] for u in uniq)
        mt = np.zeros((len(uniq), P, mw_max), dtype=_bf)
        for i, u in enumerate(uniq):
            mt[i, :, :u.shape[1]] = u
    in_maps = []
    for c in range(NC):
        qcols = np.concatenate(
            [HD * (HL * c + h) + perm_cols for h in range(HL)])
        wq_c = np.ascontiguousarray(wq[:, qcols]).astype(_bf)
        wkv_c = np.concatenate(
            [wk[:, HD * c + perm_cols], wv[:, HD * c:HD * (c + 1)]],
            axis=1).astype(_bf)
        m = {"xt": xt, "wq": wq_c, "wkv": np.ascontiguousarray(wkv_c),
             "wo": wo_b, "cos2": cos2, "sin2": sin2, "ident": ident,
             "perm": sw}
        if mt is not None:
            m["mtiles"] = mt
        in_maps.append(m)
    return in_maps


def run(x, freqs_cos, freqs_sin, mask, wq, wk, wv, wo, trace=False):
    x = np.asarray(x, dtype=np.float32)
    mask = np.asarray(mask, dtype=np.float32)
    nc, kinds, uniq = _get_compiled(mask)
    in_maps = _host_prep(x, np.asarray(freqs_cos), np.asarray(freqs_sin),
                         mask, np.asarray(wq), np.asarray(wk),
                         np.asarray(wv), np.asarray(wo), uniq)
    res = run_bass_kernel_spmd(nc, in_maps, core_ids=list(range(NC)),
                               trace=trace)
    out = np.concatenate([res.results[c]["out"] for c in range(NC)], axis=0)
    return out.reshape(1, S, D).astype(np.float32), res


def kernel(x, freqs_cos, freqs_sin, mask, wq, wk, wv, wo):
    out, _ = run(x, freqs_cos, freqs_sin, mask, wq, wk, wv, wo, trace=False)
    return out


# revision 31
# speedup vs baseline: 1.0964x; 1.0368x over previous
"""Trainium2 Bass kernel for GQA attention (B=1, S=2048, D=2048, 32 Q heads,
8 KV heads, head_dim 64), 8-way tensor parallel over heads.

Strategy (SPMD, one graph on all 8 cores):
  - Core c owns Q heads 4c..4c+3 and KV head c (GQA maps exactly).
  - Host prep: x transposed to model-dim-major bf16, chunked [128,1024] so
    projections start after ~2MB lands; weight slices bf16; RoPE pairs
    de-interleaved via weight-column permutation; cos/sin tables and windowed
    multiplicative mask tiles built from the actual mask input.
  - Projections n-chunk-outer with attention for q-group g=n interleaved in
    emission order so exp (ScalarE) overlaps projection matmuls (PE).
  - RoPE partition block-swap via a PE permutation matmul (no SBUF-SBUF DMA).
  - Scores for a head pair run as two concurrent row-tiled matmuls
    (tile_position (0,0)/(64,0), K=64 each) sharing the PE array; the rot
    output layout (two heads stacked in 128 partitions) feeds this directly.
  - Causal trimming: per (g, k'-tile) only the non-fully-masked q-column
    window is computed through scores/exp/PV.
  - attn^T = [V|1]^T P^T accumulated in PSUM (ones column = softmax denom),
    scaled by reciprocal, AllToAll per head pair; wo accumulates both halves
    into PSUM held across the second A2A.
  - DMA issue is spread across engine queues: bulk x + wo on sync (wo slots
    alias the consumed xt slots), weights on scalar/vector, a2a traffic and
    consts on gpsimd, outputs on vector.
"""

import os
import sys

import numpy as np

for _p in ("/opt/trn_rl_repo", "/root/.axon_site/_ro/trn_rl_repo"):
    if os.path.isdir(_p) and _p not in sys.path:
        sys.path.insert(0, _p)

import ml_dtypes  # noqa: E402

from concourse import bacc, mybir, tile  # noqa: E402
from concourse.bass_utils import run_bass_kernel_spmd  # noqa: E402

BF16 = mybir.dt.bfloat16
F32 = mybir.dt.float32

S = 2048          # sequence length
D = 2048          # model dim
HD = 64           # head dim
NH = 32           # query heads
NKV = 8           # kv heads
NC = 8            # cores
HL = NH // NC     # q heads per core = 4
P = 128
QG = 512          # q-group width (score-tile free dim)
NG = S // QG      # 4 q groups
NT = S // P       # 16 k'-tiles
KD = D // P       # 16 contraction tiles for D-reductions
SR = S // NC      # 256 output rows per core
XW = 1024         # xt DMA chunk width

_bf = ml_dtypes.bfloat16


def _classify_mask(mask):
    """Per (q-group g, k'-tile t) of the transposed multiplicative mask
    exp(mask)[k, q]: either skipped entirely (None), or (off, mw, u): active
    q-column window [off, 512) with a multiply window [off, off+mw) using
    unique windowed mask tile u (u = -1 if no multiply needed)."""
    mexp = np.exp(np.minimum(mask.astype(np.float64), 50.0)).astype(np.float32).T
    kinds = {}
    uniq = []
    uniq_keys = {}
    for g in range(NG):
        for t in range(NT):
            tl = mexp[P * t:P * (t + 1), QG * g:QG * (g + 1)]
            zero_col = (tl == 0.0).all(axis=0)
            one_col = (tl == 1.0).all(axis=0)
            if zero_col.all():
                kinds[(g, t)] = None
                continue
            nz = np.where(~zero_col)[0]
            off = (int(nz[0]) // 8) * 8
            ntriv = np.where(~one_col)[0]
            if len(ntriv) == 0 and off == 0:
                kinds[(g, t)] = (0, 0, -1)
                continue
            last = int(ntriv[-1]) + 1 if len(ntriv) else off + 1
            m1 = min(QG, -(-last // 8) * 8)
            win = tl[:, off:m1]
            key = (m1 - off, win.tobytes())
            if key not in uniq_keys:
                uniq_keys[key] = len(uniq)
                uniq.append(win.astype(_bf))
            kinds[(g, t)] = (off, m1 - off, uniq_keys[key])
    # the first active tile of each group must start at column 0 so its PV
    # matmul initializes the whole PSUM accumulator width
    for g in range(NG):
        for t in range(NT):
            k = kinds.get((g, t))
            if k is None:
                continue
            off, mw, u = k
            if off != 0:
                tl = mexp[P * t:P * (t + 1), QG * g:QG * (g + 1)]
                m1 = off + mw if u >= 0 else off
                m1 = max(m1, (off // 8 + 1) * 8)
                win = tl[:, 0:m1]
                key = (m1, win.tobytes())
                if key not in uniq_keys:
                    uniq_keys[key] = len(uniq)
                    uniq.append(win.astype(_bf))
                kinds[(g, t)] = (0, m1, uniq_keys[key])
            break
    return kinds, uniq


def _build_nc(kinds, uniq_widths):
    n_uniq = len(uniq_widths)
    mw_max = max(uniq_widths) if n_uniq else 0
    nc = bacc.Bacc("TRN2", target_bir_lowering=False, debug=False,
                   num_devices=NC)

    xt_d = nc.dram_tensor("xt", [D, S], BF16, kind="ExternalInput")
    wq_d = nc.dram_tensor("wq", [D, HL * HD], BF16, kind="ExternalInput")
    wkv_d = nc.dram_tensor("wkv", [D, 2 * HD], BF16, kind="ExternalInput")
    wo_d = nc.dram_tensor("wo", [D, D], BF16, kind="ExternalInput")
    cos2_d = nc.dram_tensor("cos2", [P, S], BF16, kind="ExternalInput")
    sin2_d = nc.dram_tensor("sin2", [P, S], BF16, kind="ExternalInput")
    ident_d = nc.dram_tensor("ident", [P, P], BF16, kind="ExternalInput")
    perm_d = nc.dram_tensor("perm", [P, P], BF16, kind="ExternalInput")
    mt_d = None
    if n_uniq:
        mt_d = nc.dram_tensor("mtiles", [n_uniq, P, mw_max], BF16,
                              kind="ExternalInput")
    out_d = nc.dram_tensor("out", [SR, D], BF16, kind="ExternalOutput")

    NXC = S // XW  # xt column chunks per k-slice

    with tile.TileContext(nc) as tc:
        with (
            tc.tile_pool(name="big", bufs=1) as big,        # xt chunks, then wo
            tc.tile_pool(name="wp", bufs=1) as wp,          # weights/consts
            tc.tile_pool(name="work", bufs=2) as work,
            tc.tile_pool(name="persist", bufs=1) as persist,
            tc.tile_pool(name="pt", bufs=6) as ptpool,
            tc.tile_pool(name="ps_sc", bufs=2, space="PSUM") as ps_sc,   # [128,2,512] f32 = 2 banks ea
            tc.tile_pool(name="ps_at", bufs=4, space="PSUM") as ps_at,   # 1 bank ea
            tc.tile_pool(name="dram", bufs=1, space="DRAM") as dram,
        ):
            # ---- input DMAs, spread across engine queues ----
            wq_sb = [wp.tile([P, HL * HD], BF16, tag=f"wq{k}", name=f"wq{k}")
                     for k in range(KD)]
            wkv_sb = [wp.tile([P, 2 * HD], BF16, tag=f"wkv{k}", name=f"wkv{k}")
                      for k in range(KD)]
            for k in range(KD):
                nc.gpsimd.dma_start(wkv_sb[k][:], wkv_d.ap()[P * k:P * (k + 1), :])
            cos2 = wp.tile([P, S], BF16)
            sin2 = wp.tile([P, S], BF16)
            ident = wp.tile([P, P], BF16)
            perm = wp.tile([P, P], BF16)
            nc.gpsimd.dma_start(cos2[:], cos2_d.ap())
            nc.gpsimd.dma_start(sin2[:], sin2_d.ap())
            nc.gpsimd.dma_start(ident[:], ident_d.ap())
            nc.gpsimd.dma_start(perm[:], perm_d.ap())
            mt = None
            if n_uniq:
                mt = wp.tile([P, n_uniq, 2, mw_max], BF16)
                for u in range(n_uniq):
                    # duplicated per head-of-pair for one-shot TT masks
                    for h2 in range(2):
                        nc.gpsimd.dma_start(mt[:, u, h2, 0:uniq_widths[u]],
                                            mt_d.ap()[u])
            # xt chunks split across sync/scalar queues, n-major so chunk 0
            # lands first; wq joins scalar after the first xt wave
            xt_sb = [[big.tile([P, XW], BF16, tag=f"big{NXC * k + j}",
                               name=f"x{k}_{j}")
                      for j in range(NXC)] for k in range(KD)]

            def xt_load(j):
                for k in range(KD):
                    eng = nc.sync if k % 2 == 0 else nc.scalar
                    eng.dma_start(xt_sb[k][j][:],
                                  xt_d.ap()[P * k:P * (k + 1),
                                            XW * j:XW * (j + 1)])

            xt_load(0)
            for k in range(KD):
                nc.scalar.dma_start(wq_sb[k][:], wq_d.ap()[P * k:P * (k + 1), :])
            xt_load(1)
            # wo reuses the xt slots (slot k waits for xt tile k's release)
            wo_sb = [big.tile([P, D], BF16, tag=f"big{NXC * k}", name=f"wo{k}")
                     for k in range(KD)]
            for k in range(KD):
                nc.sync.dma_start(wo_sb[k][:], wo_d.ap()[P * k:P * (k + 1), :])

            # ---- persistent activations ----
            qrot = persist.tile([P, 2, S], BF16)   # pair m: rows 0-63 head 2m, 64-127 head 2m+1
            krot2 = persist.tile([P, S], BF16)     # K rot duplicated on both halves
            vt_sb = persist.tile([HD, S], BF16)
            v_sb = persist.tile([P, NT, HD + 2], BF16)  # col 64 = ones
            nc.vector.memset(v_sb[:, :, HD:HD + 2], 1.0)

            a2a_in = [dram.tile([NC, 2 * HD, SR], BF16, tag=f"a2ai{i}",
                                name=f"a2ai{i}") for i in range(2)]
            a2a_out = [dram.tile([NC, 2 * HD, SR], BF16, tag=f"a2ao{i}",
                                 name=f"a2ao{i}") for i in range(2)]

            def proj_chunk(n, m):
                """Projection + RoPE for column chunk n, output block m
                (m=0,1: q head pairs; m=2: K|V)."""
                nsl = slice(QG * n, QG * (n + 1))
                j, joff = (QG * n) // XW, (QG * n) % XW
                ps = ps_sc.tile([P, 2, QG], F32, tag="sc", name=f"pj{n}{m}")
                for k in range(KD):
                    lhsT = (wq_sb[k][:, P * m:P * (m + 1)] if m < 2
                            else wkv_sb[k][:])
                    nc.tensor.matmul(ps[:, 0, :], lhsT,
                                     xt_sb[k][j][:, joff:joff + QG],
                                     start=(k == 0), stop=(k == KD - 1))
                raw = work.tile([P, QG], BF16, tag="raw")
                nc.vector.tensor_copy(raw[:], ps[:, 0, :])
                # partition block-swap via PE permutation matmul, into the
                # second bank of the same score slot
                nc.tensor.matmul(ps[:, 1, :], perm[:], raw[:],
                                 start=True, stop=True)
                t1 = work.tile([P, QG], BF16, tag="t1")
                nc.vector.tensor_mul(t1[:], raw[:], cos2[:, nsl])
                t2 = work.tile([P, QG], BF16, tag="t2")
                nc.vector.tensor_mul(t2[:], ps[:, 1, :], sin2[:, nsl])
                if m < 2:
                    nc.vector.tensor_add(qrot[:, m, nsl], t1[:], t2[:])
                else:
                    rot = work.tile([P, QG], BF16, tag="rot")
                    nc.vector.tensor_add(rot[:], t1[:], t2[:])
                    nc.vector.tensor_copy(krot2[0:HD, nsl], rot[0:HD, :])
                    # duplicate K rot onto partitions 64-127
                    nc.gpsimd.dma_start(krot2[HD:P, nsl], rot[0:HD, :])
                    nc.scalar.dma_start(vt_sb[:, nsl], raw[HD:P, :])
                    for t in range(4 * n, 4 * n + 4):
                        pv = ps_at.tile([P, HD], BF16, tag="at", name=f"pv{t}")
                        nc.tensor.transpose(pv[:], vt_sb[:, P * t:P * (t + 1)],
                                            ident[0:HD, 0:HD])
                        nc.vector.tensor_copy(v_sb[:, t, 0:HD], pv[:])

            def attn_group(hp, g):
                """Attention for head pair hp (heads 2hp, 2hp+1) on q-group g."""
                tiles = [t for t in range(NT) if kinds.get((g, t)) is not None]
                att = [ps_at.tile([HD + 1, QG], F32, tag="at",
                                  name=f"att{hp}{g}{h2}") for h2 in range(2)]
                qb = QG * g
                first = True
                for ci, t in enumerate(tiles):
                    off, mw, u = kinds[(g, t)]
                    w = QG - off
                    psc = ps_sc.tile([P, 2, QG], F32, tag="sc",
                                     name=f"sc{hp}{g}{t}")
                    for h2 in range(2):
                        nc.tensor.matmul(
                            psc[:, h2, 0:w],
                            krot2[HD * h2:HD * (h2 + 1), P * t:P * (t + 1)],
                            qrot[HD * h2:HD * (h2 + 1), hp, qb + off:qb + QG],
                            start=True, stop=True)
                    pts = ptpool.tile([P, 2, QG], BF16, tag="pt")
                    nc.scalar.activation(pts[:, :, 0:w], psc[:, :, 0:w],
                                         mybir.ActivationFunctionType.Exp,
                                         scale=0.125)
                    if u >= 0:
                        nc.vector.tensor_mul(pts[:, :, 0:mw], pts[:, :, 0:mw],
                                             mt[:, u, :, 0:mw])
                    for h2 in range(2):
                        nc.tensor.matmul(att[h2][:, off:QG],
                                         v_sb[:, t, 0:HD + 1],
                                         pts[:, h2, 0:w],
                                         start=first,
                                         stop=(ci == len(tiles) - 1),
                                         skip_group_check=True)
                    first = False
                for h2 in range(2):
                    den = work.tile([1, QG], F32, tag="den")
                    nc.vector.tensor_copy(den[:], att[h2][HD:HD + 1, :])
                    rec = work.tile([1, QG], F32, tag="rec")
                    nc.vector.reciprocal_approx_fast(rec[:], den[:])
                    rec64 = work.tile([HD, QG], F32, tag="rec64")
                    nc.gpsimd.partition_broadcast(rec64[:], rec[:])
                    asb = work.tile([HD, QG], BF16, tag="asb")
                    nc.vector.tensor_mul(asb[:], att[h2][0:HD, :], rec64[:])
                    hr = HD * h2
                    nc.gpsimd.dma_start(a2a_in[hp][2 * g, hr:hr + HD, :],
                                        asb[:, 0:SR])
                    nc.gpsimd.dma_start(a2a_in[hp][2 * g + 1, hr:hr + HD, :],
                                        asb[:, SR:2 * SR])

            # ---- emission: proj chunk n with attention; head pair 1 lags
            # two q-groups so A2A#1 hides under hp1's tail ----
            for n in range(NG):
                for m in (2, 0, 1):
                    proj_chunk(n, m)
                attn_group(0, n)
                if n >= 2:
                    attn_group(1, n - 2)
            nc.gpsimd.collective_compute(
                "AllToAll", mybir.AluOpType.bypass,
                replica_groups=[list(range(NC))],
                ins=[a2a_in[0].opt()], outs=[a2a_out[0].opt()])
            attn_group(1, NG - 2)
            attn_group(1, NG - 1)
            nc.gpsimd.collective_compute(
                "AllToAll", mybir.AluOpType.bypass,
                replica_groups=[list(range(NC))],
                ins=[a2a_in[1].opt()], outs=[a2a_out[1].opt()])

            # ---- output: out[s_rows, :] = attnT_full^T @ wo ----
            # a2a_out[half] block i = wo k-tile 2i+half
            ao_sb = [[persist.tile([P, SR], BF16, tag=f"ao{h}_{i}",
                                   name=f"ao{h}_{i}") for i in range(NC)]
                     for h in range(2)]
            for h in range(2):
                for i in range(NC):
                    nc.gpsimd.dma_start(ao_sb[h][i][:], a2a_out[h][i])

            # 8 accumulators [128, 512] over all 8 PSUM banks; half 0 (even
            # wo k-tiles, A2A#1 data) accumulates while A2A#2 is in flight.
            NE = D // QG
            acc = {}
            for sm in range(SR // P):
                pscl = ps_sc.tile([P, 2, QG], F32, tag="sc", name=f"oacc{sm}")
                acc[(sm, 0)] = pscl[:, 0, :]
                acc[(sm, 1)] = pscl[:, 1, :]
                for e2 in range(2):
                    a = ps_at.tile([P, QG], F32, tag="at", name=f"oat{sm}{e2}")
                    acc[(sm, 2 + e2)] = a[:]
            engs = [nc.scalar, nc.sync, nc.gpsimd]
            for sm in range(SR // P):
                for ec in range(NE):
                    for i in range(NC):
                        nc.tensor.matmul(
                            acc[(sm, ec)],
                            ao_sb[0][i][:, P * sm:P * (sm + 1)],
                            wo_sb[2 * i][:, QG * ec:QG * (ec + 1)],
                            start=(i == 0), stop=False,
                            skip_group_check=True)
            for i2, (sm, ec) in enumerate([(s, e) for s in range(SR // P)
                                           for e in range(NE)]):
                for i in range(NC):
                    nc.tensor.matmul(
                        acc[(sm, ec)],
                        ao_sb[1][i][:, P * sm:P * (sm + 1)],
                        wo_sb[2 * i + 1][:, QG * ec:QG * (ec + 1)],
                        start=False, stop=(i == NC - 1),
                        skip_group_check=True)
                osb = work.tile([P, QG], BF16, tag="osb")
                nc.vector.tensor_copy(osb[:], acc[(sm, ec)])
                engs[i2 % 3].dma_start(
                    out_d.ap()[P * sm:P * (sm + 1),
                               QG * ec:QG * (ec + 1)],
                    osb[:])

    nc.compile()
    return nc


_CACHE = {}


def _get_compiled(mask):
    kinds, uniq = _classify_mask(mask)
    key = tuple(sorted(kinds.items())) + tuple(u.tobytes() for u in uniq)
    if key not in _CACHE:
        _CACHE[key] = _build_nc(kinds, [u.shape[1] for u in uniq])
    return _CACHE[key], kinds, uniq


def _host_prep(x, freqs_cos, freqs_sin, mask, wq, wk, wv, wo, uniq):
    xt = np.ascontiguousarray(x[0].T).astype(_bf)
    perm_cols = np.concatenate([np.arange(0, HD, 2), np.arange(1, HD, 2)])
    cosT = np.ascontiguousarray(freqs_cos.T)            # [32, S]
    sinT = np.ascontiguousarray(freqs_sin.T)
    cos2 = np.tile(cosT, (4, 1)).astype(_bf)            # [128, S]
    sin2 = np.tile(np.concatenate([-sinT, sinT], axis=0), (2, 1)).astype(_bf)
    ident = np.eye(P, dtype=_bf)
    # block-swap permutation matrix: swap the 32-halves within each 64 block
    sw = np.zeros((P, P), dtype=_bf)
    for b in range(4):
        d0, s0 = 32 * b, 32 * (b ^ 1)
        sw[s0:s0 + 32, d0:d0 + 32] = np.eye(32, dtype=_bf)
    wo_b = np.ascontiguousarray(wo).astype(_bf)
    mt = None
    if uniq:
        mw_max = max(u.shape[1] for u in uniq)
        mt = np.zeros((len(uniq), P, mw_max), dtype=_bf)
        for i, u in enumerate(uniq):
            mt[i, :, :u.shape[1]] = u
    in_maps = []
    for c in range(NC):
        qcols = np.concatenate(
            [HD * (HL * c + h) + perm_cols for h in range(HL)])
        wq_c = np.ascontiguousarray(wq[:, qcols]).astype(_bf)
        wkv_c = np.concatenate(
            [wk[:, HD * c + perm_cols], wv[:, HD * c:HD * (c + 1)]],
            axis=1).astype(_bf)
        m = {"xt": xt, "wq": wq_c, "wkv": np.ascontiguousarray(wkv_c),
             "wo": wo_b, "cos2": cos2, "sin2": sin2, "ident": ident,
             "perm": sw}
        if mt is not None:
            m["mtiles"] = mt
        in_maps.append(m)
    return in_maps


def run(x, freqs_cos, freqs_sin, mask, wq, wk, wv, wo, trace=False):
    x = np.asarray(x, dtype=np.float32)
    mask = np.asarray(mask, dtype=np.float32)
    nc, kinds, uniq = _get_compiled(mask)
    in_maps = _host_prep(x, np.asarray(freqs_cos), np.asarray(freqs_sin),
                         mask, np.asarray(wq), np.asarray(wk),
                         np.asarray(wv), np.asarray(wo), uniq)
    res = run_bass_kernel_spmd(nc, in_maps, core_ids=list(range(NC)),
                               trace=trace)
    out = np.concatenate([res.results[c]["out"] for c in range(NC)], axis=0)
    return out.reshape(1, S, D).astype(np.float32), res


def kernel(x, freqs_cos, freqs_sin, mask, wq, wk, wv, wo):
    out, _ = run(x, freqs_cos, freqs_sin, mask, wq, wk, wv, wo, trace=False)
    return out


# revision 32
# speedup vs baseline: 1.1227x; 1.0240x over previous
"""Trainium2 Bass kernel for GQA attention (B=1, S=2048, D=2048, 32 Q heads,
8 KV heads, head_dim 64), 8-way tensor parallel over heads.

Strategy (SPMD, one graph on all 8 cores):
  - Core c owns Q heads 4c..4c+3 and KV head c (GQA maps exactly).
  - Host prep: x transposed to model-dim-major bf16, chunked [128,1024] so
    projections start after ~2MB lands; weight slices bf16; RoPE pairs
    de-interleaved via weight-column permutation; cos/sin tables and windowed
    multiplicative mask tiles built from the actual mask input.
  - Projections n-chunk-outer with attention for q-group g=n interleaved in
    emission order so exp (ScalarE) overlaps projection matmuls (PE).
  - RoPE partition block-swap via a PE permutation matmul (no SBUF-SBUF DMA).
  - Scores for a head pair run as two concurrent row-tiled matmuls
    (tile_position (0,0)/(64,0), K=64 each) sharing the PE array; the rot
    output layout (two heads stacked in 128 partitions) feeds this directly.
  - Causal trimming: per (g, k'-tile) only the non-fully-masked q-column
    window is computed through scores/exp/PV.
  - attn^T = [V|1]^T P^T accumulated in PSUM (ones column = softmax denom),
    scaled by reciprocal, AllToAll per head pair; wo accumulates both halves
    into PSUM held across the second A2A.
  - DMA issue is spread across engine queues: bulk x + wo on sync (wo slots
    alias the consumed xt slots), weights on scalar/vector, a2a traffic and
    consts on gpsimd, outputs on vector.
"""

import os
import sys

import numpy as np

for _p in ("/opt/trn_rl_repo", "/root/.axon_site/_ro/trn_rl_repo"):
    if os.path.isdir(_p) and _p not in sys.path:
        sys.path.insert(0, _p)

import ml_dtypes  # noqa: E402

from concourse import bacc, mybir, tile  # noqa: E402
from concourse.bass_utils import run_bass_kernel_spmd  # noqa: E402

BF16 = mybir.dt.bfloat16
F32 = mybir.dt.float32

S = 2048          # sequence length
D = 2048          # model dim
HD = 64           # head dim
NH = 32           # query heads
NKV = 8           # kv heads
NC = 8            # cores
HL = NH // NC     # q heads per core = 4
P = 128
QG = 512          # q-group width (score-tile free dim)
NG = S // QG      # 4 q groups
NT = S // P       # 16 k'-tiles
KD = D // P       # 16 contraction tiles for D-reductions
SR = S // NC      # 256 output rows per core
XW = 1024         # xt DMA chunk width

_bf = ml_dtypes.bfloat16


def _classify_mask(mask):
    """Per (q-group g, k'-tile t) of the transposed multiplicative mask
    exp(mask)[k, q]: either skipped entirely (None), or (off, mw, u): active
    q-column window [off, 512) with a multiply window [off, off+mw) using
    unique windowed mask tile u (u = -1 if no multiply needed)."""
    mexp = np.exp(np.minimum(mask.astype(np.float64), 50.0)).astype(np.float32).T
    kinds = {}
    uniq = []
    uniq_keys = {}
    for g in range(NG):
        for t in range(NT):
            tl = mexp[P * t:P * (t + 1), QG * g:QG * (g + 1)]
            zero_col = (tl == 0.0).all(axis=0)
            one_col = (tl == 1.0).all(axis=0)
            if zero_col.all():
                kinds[(g, t)] = None
                continue
            nz = np.where(~zero_col)[0]
            off = (int(nz[0]) // 8) * 8
            ntriv = np.where(~one_col)[0]
            if len(ntriv) == 0 and off == 0:
                kinds[(g, t)] = (0, 0, -1)
                continue
            last = int(ntriv[-1]) + 1 if len(ntriv) else off + 1
            m1 = min(QG, -(-last // 8) * 8)
            win = tl[:, off:m1]
            key = (m1 - off, win.tobytes())
            if key not in uniq_keys:
                uniq_keys[key] = len(uniq)
                uniq.append(win.astype(_bf))
            kinds[(g, t)] = (off, m1 - off, uniq_keys[key])
    # the first active tile of each group must start at column 0 so its PV
    # matmul initializes the whole PSUM accumulator width
    for g in range(NG):
        for t in range(NT):
            k = kinds.get((g, t))
            if k is None:
                continue
            off, mw, u = k
            if off != 0:
                tl = mexp[P * t:P * (t + 1), QG * g:QG * (g + 1)]
                m1 = off + mw if u >= 0 else off
                m1 = max(m1, (off // 8 + 1) * 8)
                win = tl[:, 0:m1]
                key = (m1, win.tobytes())
                if key not in uniq_keys:
                    uniq_keys[key] = len(uniq)
                    uniq.append(win.astype(_bf))
                kinds[(g, t)] = (0, m1, uniq_keys[key])
            break
    return kinds, uniq


def _build_nc(kinds, uniq_widths):
    n_uniq = len(uniq_widths)
    mw_max = max(uniq_widths) if n_uniq else 0
    nc = bacc.Bacc("TRN2", target_bir_lowering=False, debug=False,
                   num_devices=NC)

    xt_d = nc.dram_tensor("xt", [D, S], BF16, kind="ExternalInput")
    wq_d = nc.dram_tensor("wq", [D, HL * HD], BF16, kind="ExternalInput")
    wkv_d = nc.dram_tensor("wkv", [D, 2 * HD], BF16, kind="ExternalInput")
    wo_d = nc.dram_tensor("wo", [D, D], BF16, kind="ExternalInput")
    cos2_d = nc.dram_tensor("cos2", [P, S], BF16, kind="ExternalInput")
    sin2_d = nc.dram_tensor("sin2", [P, S], BF16, kind="ExternalInput")
    ident_d = nc.dram_tensor("ident", [P, P], BF16, kind="ExternalInput")
    perm_d = nc.dram_tensor("perm", [P, P], BF16, kind="ExternalInput")
    mt_d = None
    if n_uniq:
        mt_d = nc.dram_tensor("mtiles", [n_uniq, P, mw_max], BF16,
                              kind="ExternalInput")
    out_d = nc.dram_tensor("out", [SR, D], BF16, kind="ExternalOutput")

    NXC = S // XW  # xt column chunks per k-slice

    with tile.TileContext(nc) as tc:
        with (
            tc.tile_pool(name="big", bufs=1) as big,        # xt chunks, then wo
            tc.tile_pool(name="wp", bufs=1) as wp,          # weights/consts
            tc.tile_pool(name="work", bufs=2) as work,
            tc.tile_pool(name="persist", bufs=1) as persist,
            tc.tile_pool(name="pt", bufs=6) as ptpool,
            tc.tile_pool(name="ps_sc", bufs=2, space="PSUM") as ps_sc,   # [128,2,512] f32 = 2 banks ea
            tc.tile_pool(name="ps_at", bufs=4, space="PSUM") as ps_at,   # 1 bank ea
            tc.tile_pool(name="dram", bufs=1, space="DRAM") as dram,
        ):
            # ---- input DMAs, spread across engine queues ----
            wq_sb = [wp.tile([P, HL * HD], BF16, tag=f"wq{k}", name=f"wq{k}")
                     for k in range(KD)]
            wkv_sb = [wp.tile([P, 2 * HD], BF16, tag=f"wkv{k}", name=f"wkv{k}")
                      for k in range(KD)]
            for k in range(KD):
                nc.gpsimd.dma_start(wkv_sb[k][:], wkv_d.ap()[P * k:P * (k + 1), :])
            cos2 = wp.tile([P, S], BF16)
            sin2 = wp.tile([P, S], BF16)
            ident = wp.tile([P, P], BF16)
            perm = wp.tile([P, P], BF16)
            nc.gpsimd.dma_start(cos2[:], cos2_d.ap())
            nc.gpsimd.dma_start(sin2[:], sin2_d.ap())
            nc.gpsimd.dma_start(ident[:], ident_d.ap())
            nc.gpsimd.dma_start(perm[:], perm_d.ap())
            mt = None
            if n_uniq:
                mt = wp.tile([P, n_uniq, 2, mw_max], BF16)
                for u in range(n_uniq):
                    # duplicated per head-of-pair for one-shot TT masks
                    for h2 in range(2):
                        nc.gpsimd.dma_start(mt[:, u, h2, 0:uniq_widths[u]],
                                            mt_d.ap()[u])
            # xt chunks split across sync/scalar queues, n-major so chunk 0
            # lands first; wq joins scalar after the first xt wave
            xt_sb = [[big.tile([P, XW], BF16, tag=f"big{NXC * k + j}",
                               name=f"x{k}_{j}")
                      for j in range(NXC)] for k in range(KD)]

            def xt_load(j):
                for k in range(KD):
                    eng = nc.sync if k % 2 == 0 else nc.scalar
                    eng.dma_start(xt_sb[k][j][:],
                                  xt_d.ap()[P * k:P * (k + 1),
                                            XW * j:XW * (j + 1)])

            xt_load(0)
            for k in range(KD):
                nc.scalar.dma_start(wq_sb[k][:], wq_d.ap()[P * k:P * (k + 1), :])
            xt_load(1)
            # wo reuses the xt slots (slot k waits for xt tile k's release)
            wo_sb = [big.tile([P, D], BF16, tag=f"big{NXC * k}", name=f"wo{k}")
                     for k in range(KD)]
            for k in range(KD):
                nc.sync.dma_start(wo_sb[k][:], wo_d.ap()[P * k:P * (k + 1), :])

            # ---- persistent activations ----
            qrot = persist.tile([P, 2, S], BF16)   # pair m: rows 0-63 head 2m, 64-127 head 2m+1
            krot2 = persist.tile([P, S], BF16)     # K rot duplicated on both halves
            vt_sb = persist.tile([HD, S], BF16)
            v_sb = persist.tile([P, NT, HD + 2], BF16)  # col 64 = ones
            nc.vector.memset(v_sb[:, :, HD:HD + 2], 1.0)

            a2a_in = [dram.tile([NC, 2 * HD, SR], BF16, tag=f"a2ai{i}",
                                name=f"a2ai{i}") for i in range(2)]
            a2a_out = [dram.tile([NC, 2 * HD, SR], BF16, tag=f"a2ao{i}",
                                 name=f"a2ao{i}") for i in range(2)]

            def proj_chunk(n, m):
                """Projection + RoPE for column chunk n, output block m
                (m=0,1: q head pairs; m=2: K|V)."""
                nsl = slice(QG * n, QG * (n + 1))
                j, joff = (QG * n) // XW, (QG * n) % XW
                ps = ps_sc.tile([P, 2, QG], F32, tag="sc", name=f"pj{n}{m}")
                for k in range(KD):
                    lhsT = (wq_sb[k][:, P * m:P * (m + 1)] if m < 2
                            else wkv_sb[k][:])
                    nc.tensor.matmul(ps[:, 0, :], lhsT,
                                     xt_sb[k][j][:, joff:joff + QG],
                                     start=(k == 0), stop=(k == KD - 1))
                raw = work.tile([P, QG], BF16, tag="raw")
                nc.vector.tensor_copy(raw[:], ps[:, 0, :])
                # partition block-swap via PE permutation matmul, into the
                # second bank of the same score slot
                nc.tensor.matmul(ps[:, 1, :], perm[:], raw[:],
                                 start=True, stop=True)
                t1 = work.tile([P, QG], BF16, tag="t1")
                nc.vector.tensor_mul(t1[:], raw[:], cos2[:, nsl])
                t2 = work.tile([P, QG], BF16, tag="t2")
                nc.vector.tensor_mul(t2[:], ps[:, 1, :], sin2[:, nsl])
                if m < 2:
                    nc.vector.tensor_add(qrot[:, m, nsl], t1[:], t2[:])
                else:
                    rot = work.tile([P, QG], BF16, tag="rot")
                    nc.vector.tensor_add(rot[:], t1[:], t2[:])
                    nc.vector.tensor_copy(krot2[0:HD, nsl], rot[0:HD, :])
                    # duplicate K rot onto partitions 64-127
                    nc.gpsimd.dma_start(krot2[HD:P, nsl], rot[0:HD, :])
                    nc.scalar.dma_start(vt_sb[:, nsl], raw[HD:P, :])
                    for t in range(4 * n, 4 * n + 4):
                        pv = ps_at.tile([P, HD], BF16, tag="at", name=f"pv{t}")
                        nc.tensor.transpose(pv[:], vt_sb[:, P * t:P * (t + 1)],
                                            ident[0:HD, 0:HD])
                        nc.vector.tensor_copy(v_sb[:, t, 0:HD], pv[:])

            def attn_group(hp, g):
                """Attention for head pair hp (heads 2hp, 2hp+1) on q-group g."""
                tiles = [t for t in range(NT) if kinds.get((g, t)) is not None]
                att = [ps_at.tile([HD + 1, QG], F32, tag="at",
                                  name=f"att{hp}{g}{h2}") for h2 in range(2)]
                qb = QG * g
                first = True
                for ci, t in enumerate(tiles):
                    off, mw, u = kinds[(g, t)]
                    w = QG - off
                    psc = ps_sc.tile([P, 2, QG], F32, tag="sc",
                                     name=f"sc{hp}{g}{t}")
                    for h2 in range(2):
                        nc.tensor.matmul(
                            psc[:, h2, 0:w],
                            krot2[HD * h2:HD * (h2 + 1), P * t:P * (t + 1)],
                            qrot[HD * h2:HD * (h2 + 1), hp, qb + off:qb + QG],
                            start=True, stop=True)
                    pts = ptpool.tile([P, 2, QG], BF16, tag="pt")
                    nc.scalar.activation(pts[:, :, 0:w], psc[:, :, 0:w],
                                         mybir.ActivationFunctionType.Exp,
                                         scale=0.125)
                    if u >= 0:
                        nc.vector.tensor_mul(pts[:, :, 0:mw], pts[:, :, 0:mw],
                                             mt[:, u, :, 0:mw])
                    for h2 in range(2):
                        nc.tensor.matmul(att[h2][:, off:QG],
                                         v_sb[:, t, 0:HD + 1],
                                         pts[:, h2, 0:w],
                                         start=first,
                                         stop=(ci == len(tiles) - 1),
                                         skip_group_check=True)
                    first = False
                for h2 in range(2):
                    den = work.tile([1, QG], F32, tag="den")
                    nc.vector.tensor_copy(den[:], att[h2][HD:HD + 1, :])
                    rec = work.tile([1, QG], F32, tag="rec")
                    nc.vector.reciprocal_approx_fast(rec[:], den[:])
                    rec64 = work.tile([HD, QG], F32, tag="rec64")
                    nc.gpsimd.partition_broadcast(rec64[:], rec[:])
                    asb = work.tile([HD, QG], BF16, tag="asb")
                    nc.vector.tensor_mul(asb[:], att[h2][0:HD, :], rec64[:])
                    hr = HD * h2
                    nc.gpsimd.dma_start(a2a_in[hp][2 * g, hr:hr + HD, :],
                                        asb[:, 0:SR])
                    nc.gpsimd.dma_start(a2a_in[hp][2 * g + 1, hr:hr + HD, :],
                                        asb[:, SR:2 * SR])

            # ---- emission: proj chunk n with attention; head pair 1 lags
            # one q-group so A2A#1 hides under hp1's tail ----
            for n in range(NG):
                for m in (2, 0, 1):
                    proj_chunk(n, m)
                attn_group(0, n)
                if n == NG - 1:
                    nc.gpsimd.collective_compute(
                        "AllToAll", mybir.AluOpType.bypass,
                        replica_groups=[list(range(NC))],
                        ins=[a2a_in[0].opt()], outs=[a2a_out[0].opt()])
                if n >= 1:
                    attn_group(1, n - 1)
            attn_group(1, NG - 1)
            nc.gpsimd.collective_compute(
                "AllToAll", mybir.AluOpType.bypass,
                replica_groups=[list(range(NC))],
                ins=[a2a_in[1].opt()], outs=[a2a_out[1].opt()])

            # ---- output: out[s_rows, :] = attnT_full^T @ wo ----
            # a2a_out[half] block i = wo k-tile 2i+half
            ao_sb = [[persist.tile([P, SR], BF16, tag=f"ao{h}_{i}",
                                   name=f"ao{h}_{i}") for i in range(NC)]
                     for h in range(2)]
            for h in range(2):
                for i in range(NC):
                    nc.gpsimd.dma_start(ao_sb[h][i][:], a2a_out[h][i])

            # 8 accumulators [128, 512] over all 8 PSUM banks; half 0 (even
            # wo k-tiles, A2A#1 data) accumulates while A2A#2 is in flight.
            NE = D // QG
            acc = {}
            for sm in range(SR // P):
                pscl = ps_sc.tile([P, 2, QG], F32, tag="sc", name=f"oacc{sm}")
                acc[(sm, 0)] = pscl[:, 0, :]
                acc[(sm, 1)] = pscl[:, 1, :]
                for e2 in range(2):
                    a = ps_at.tile([P, QG], F32, tag="at", name=f"oat{sm}{e2}")
                    acc[(sm, 2 + e2)] = a[:]
            engs = [nc.scalar, nc.sync, nc.gpsimd]
            for sm in range(SR // P):
                for ec in range(NE):
                    for i in range(NC):
                        nc.tensor.matmul(
                            acc[(sm, ec)],
                            ao_sb[0][i][:, P * sm:P * (sm + 1)],
                            wo_sb[2 * i][:, QG * ec:QG * (ec + 1)],
                            start=(i == 0), stop=False,
                            skip_group_check=True)
            for i2, (sm, ec) in enumerate([(s, e) for s in range(SR // P)
                                           for e in range(NE)]):
                for i in range(NC):
                    nc.tensor.matmul(
                        acc[(sm, ec)],
                        ao_sb[1][i][:, P * sm:P * (sm + 1)],
                        wo_sb[2 * i + 1][:, QG * ec:QG * (ec + 1)],
                        start=False, stop=(i == NC - 1),
                        skip_group_check=True)
                osb = work.tile([P, QG], BF16, tag="osb")
                nc.vector.tensor_copy(osb[:], acc[(sm, ec)])
                engs[i2 % 3].dma_start(
                    out_d.ap()[P * sm:P * (sm + 1),
                               QG * ec:QG * (ec + 1)],
                    osb[:])

    nc.compile()
    return nc


_CACHE = {}


def _get_compiled(mask):
    kinds, uniq = _classify_mask(mask)
    key = tuple(sorted(kinds.items())) + tuple(u.tobytes() for u in uniq)
    if key not in _CACHE:
        _CACHE[key] = _build_nc(kinds, [u.shape[1] for u in uniq])
    return _CACHE[key], kinds, uniq


def _host_prep(x, freqs_cos, freqs_sin, mask, wq, wk, wv, wo, uniq):
    xt = np.ascontiguousarray(x[0].T).astype(_bf)
    perm_cols = np.concatenate([np.arange(0, HD, 2), np.arange(1, HD, 2)])
    cosT = np.ascontiguousarray(freqs_cos.T)            # [32, S]
    sinT = np.ascontiguousarray(freqs_sin.T)
    cos2 = np.tile(cosT, (4, 1)).astype(_bf)            # [128, S]
    sin2 = np.tile(np.concatenate([-sinT, sinT], axis=0), (2, 1)).astype(_bf)
    ident = np.eye(P, dtype=_bf)
    # block-swap permutation matrix: swap the 32-halves within each 64 block
    sw = np.zeros((P, P), dtype=_bf)
    for b in range(4):
        d0, s0 = 32 * b, 32 * (b ^ 1)
        sw[s0:s0 + 32, d0:d0 + 32] = np.eye(32, dtype=_bf)
    wo_b = np.ascontiguousarray(wo).astype(_bf)
    mt = None
    if uniq:
        mw_max = max(u.shape[1] for u in uniq)
        mt = np.zeros((len(uniq), P, mw_max), dtype=_bf)
        for i, u in enumerate(uniq):
            mt[i, :, :u.shape[1]] = u
    in_maps = []
    for c in range(NC):
        qcols = np.concatenate(
            [HD * (HL * c + h) + perm_cols for h in range(HL)])
        wq_c = np.ascontiguousarray(wq[:, qcols]).astype(_bf)
        wkv_c = np.concatenate(
            [wk[:, HD * c + perm_cols], wv[:, HD * c:HD * (c + 1)]],
            axis=1).astype(_bf)
        m = {"xt": xt, "wq": wq_c, "wkv": np.ascontiguousarray(wkv_c),
             "wo": wo_b, "cos2": cos2, "sin2": sin2, "ident": ident,
             "perm": sw}
        if mt is not None:
            m["mtiles"] = mt
        in_maps.append(m)
    return in_maps


def run(x, freqs_cos, freqs_sin, mask, wq, wk, wv, wo, trace=False):
    x = np.asarray(x, dtype=np.float32)
    mask = np.asarray(mask, dtype=np.float32)
    nc, kinds, uniq = _get_compiled(mask)
    in_maps = _host_prep(x, np.asarray(freqs_cos), np.asarray(freqs_sin),
                         mask, np.asarray(wq), np.asarray(wk),
                         np.asarray(wv), np.asarray(wo), uniq)
    res = run_bass_kernel_spmd(nc, in_maps, core_ids=list(range(NC)),
                               trace=trace)
    out = np.concatenate([res.results[c]["out"] for c in range(NC)], axis=0)
    return out.reshape(1, S, D).astype(np.float32), res


def kernel(x, freqs_cos, freqs_sin, mask, wq, wk, wv, wo):
    out, _ = run(x, freqs_cos, freqs_sin, mask, wq, wk, wv, wo, trace=False)
    return out
